# revision 2
# baseline (speedup 1.0000x reference)
"""Bahdanau-attention Trainium2 kernel (data-parallel over 8 NeuronCores).

Computation (per batch row b):
    energy[s, d] = tanh(hidden[b] @ W_h + enc[b, s] @ W_e + b_attn)   [S, D]
    scores[s]    = energy[s] . w_v                                     [S]
    attn         = softmax(scores)                                     [S]
    out[b]       = sum_s attn[s] * enc[b, s]                           [E]

v11 — batch-group col-tiled weighted sum, fused DVE chain, ACT-side
softmax sums:
  - halves processed in h-major sub-rounds of GRP=4 batches; the
    weighted sum for a sub-round runs as 8 spans of 4 col-tiled
    (tile_position=(0,32j)) M=1 matmuls — 4 batches stream
    concurrently on disjoint PE column groups, each batch's output row
    accumulating at psum partition 32j of one shared orow bank.
  - energy matmuls in fp8e4m3 DoubleRow (unchanged v10 structure);
    host stages enc*4 and W_e*8 to lift fp8 denormals, compensated
    with ACT scale=1/32 on the tanh.
  - wv multiply + running sum fused into one DVE scalar_tensor_tensor
    per dc chunk (was mul+add).
  - softmax denominator: exp accum_out gives per-partition sums free
    on ACT; GPSIMD XYZWC-reduce collapses them to [1,1] at partition
    32j; DVE reciprocal in place; one tensor_scalar scale per group.
  - psum: 3x2 banks energy + 1 orow + 1 scth = 8.
  - per-half emission: E1 E2 Wspan E3 Wspan E0' S exp, bursts spread
    2 spans per half across the following sub-round.
"""

import numpy as np

B, S, ENC, DEC = 64, 2048, 512, 512
NCORES = 8
BL = B // NCORES          # batches per core
P = 128
EC = ENC // P             # 4 e-chunks
DC = DEC // P             # 4 d-chunks
ST = 512                  # matmul moving free-dim tile / DMA block
NST = S // ST             # 4 s-blocks per batch
HT = 1024                 # psum energy tile free size (one half)
NH = S // HT              # 2 halves
NSC = S // P              # 16 s-chunks for the weighted sum
HSC = HT // P             # 8 s-chunks per half
NWARM = 64                # prewarm matmuls
G = 2                     # 256-wide DoubleRow contraction chunks
KO = 2                    # k-tiles per DoubleRow matmul
GRP = 4                   # batches per weighted-sum col-tile group
NGRP = BL // GRP
WSCALE = 8.0              # host-side W_e scale (fp8 denormal lift)
ESCALE = 4.0              # host-side enc scale for the energy copy

_PROGRAM = None


def _build_program():
    import concourse.mybir as mybir
    import concourse.tile as tile
    from concourse import bacc
    from contextlib import ExitStack

    fp32 = mybir.dt.float32
    bf16 = mybir.dt.bfloat16
    fp8 = mybir.dt.float8e4
    AF = mybir.ActivationFunctionType
    ALU = mybir.AluOpType
    AX = mybir.AxisListType

    nc = bacc.Bacc("TRN2", debug=False, target_bir_lowering=False,
                   num_devices=NCORES)

    enc4_d = nc.dram_tensor("encT4", [BL, NST, P, G, KO, ST], fp8,
                            kind="ExternalInput").ap()
    we_d = nc.dram_tensor("weT", [G, KO, P, DEC], fp8,
                          kind="ExternalInput").ap()
    encn_d = nc.dram_tensor("encN", [BL, NH, P, HSC, ENC], bf16,
                            kind="ExternalInput").ap()
    biasT_d = nc.dram_tensor("biasT", [P, DC, BL], fp32,
                             kind="ExternalInput").ap()
    wv_d = nc.dram_tensor("wv", [P, DC], fp32, kind="ExternalInput").ap()
    out_d = nc.dram_tensor("out", [BL, ENC], fp32, kind="ExternalOutput").ap()

    with tile.TileContext(nc) as tc, ExitStack() as ctx:
        const = ctx.enter_context(tc.tile_pool(name="const", bufs=1))
        # 3 energy-psum bufs (6 banks) + 1 orow bank + 1 scth bank = 8
        ps_e = ctx.enter_context(tc.tile_pool(name="ps_e", bufs=3, space="PSUM"))
        ps_or = ctx.enter_context(tc.tile_pool(name="ps_or", bufs=1, space="PSUM"))
        ps_sc = ctx.enter_context(tc.tile_pool(name="ps_sc", bufs=1, space="PSUM"))
        enc4_pool = ctx.enter_context(tc.tile_pool(name="enc4p", bufs=8))
        encn_pool = ctx.enter_context(tc.tile_pool(name="encnp", bufs=8))
        tanh_pool = ctx.enter_context(tc.tile_pool(name="tanhp", bufs=6))
        wve_pool = ctx.enter_context(tc.tile_pool(name="wvep", bufs=6))
        wvs_pool = ctx.enter_context(tc.tile_pool(name="wvsp", bufs=8))
        probs_pool = ctx.enter_context(tc.tile_pool(name="probsp", bufs=8))
        stage_pool = ctx.enter_context(tc.tile_pool(name="stagep", bufs=2))

        we_sb = const.tile([P, G, KO, DEC], fp8)
        biasT_sb = const.tile([P, DC, BL], fp32)
        wv_sb = const.tile([P, DC], fp32)
        ones_sb = const.tile([P, 1], bf16)
        warm_sb = const.tile([P, P], bf16)
        warmout_sb = const.tile([1, 1], fp32)
        acc_sb = const.tile([P, BL * NH], fp32)    # exp accum slots
        ssum_sb = const.tile([P, NGRP], fp32)      # denominators at 32j
        rs_sb = const.tile([P, NGRP], fp32)        # reciprocals at 32j

        nc.vector.memset(warm_sb[:], 1.0)
        nc.vector.memset(ones_sb[:], 1.0)
        nc.vector.memset(rs_sb[:], 1.0)
        nc.vector.memset(ssum_sb[:], 1.0)

        nc.sync.dma_start(we_sb[:], we_d.rearrange("g k p d -> p g k d"))
        nc.scalar.dma_start(biasT_sb[:], biasT_d)
        nc.scalar.dma_start(wv_sb[:], wv_d)

        # HAM prewarm: dummy accumulating matmuls, no data deps
        wps = ps_e.tile([P, P], fp32, tag="pse", name="warmps")
        for i in range(NWARM):
            nc.tensor.matmul(wps[:], lhsT=warm_sb[:], rhs=warm_sb[:],
                             start=(i == 0), stop=(i == NWARM - 1))
        nc.vector.tensor_copy(warmout_sb[:], wps[0:1, 0:1])

        enc4_t, encn_t = {}, {}
        probs_t, orow_t = {}, {}
        run_t = {}

        halves = [(gg * GRP + j, h) for gg in range(NGRP)
                  for h in range(NH) for j in range(GRP)]

        def issue_enc4(b, h):
            for st in (2 * h, 2 * h + 1):
                t = enc4_pool.tile([P, G, KO, ST], fp8, tag="enc4",
                                   name=f"enc4_{b}_{st}")
                eng = nc.scalar if st % 2 == 1 else nc.sync
                eng.dma_start(t[:], enc4_d[b, st])
                enc4_t[(b, st)] = t

        def issue_encn(b, h):
            t = encn_pool.tile([P, HSC, ENC], bf16, tag="encn",
                               name=f"encn{b}_{h}")
            nc.sync.dma_start(t[:], encn_d[b, h])
            encn_t[(b, h)] = t

        def emit_energy(b, h, dc):
            eps = ps_e.tile([P, HT], fp32, tag="pse", name=f"eps{b}_{h}_{dc}")
            for st in range(HT // ST):
                blk = h * (HT // ST) + st
                for g in range(G):
                    nc.tensor.matmul(
                        eps[:, st * ST:(st + 1) * ST],
                        lhsT=we_sb[:, g, :, dc * P:(dc + 1) * P],
                        rhs=enc4_t[(b, blk)][:, g, :, :],
                        start=(g == 0), stop=(g == G - 1),
                        perf_mode=mybir.MatmulPerfMode.DoubleRow)
            t = tanh_pool.tile([P, HT], bf16, tag="tanh",
                               name=f"tanh{b}_{h}_{dc}")
            nc.scalar.activation(t[:], eps[:], AF.Tanh,
                                 bias=biasT_sb[:, dc, b:b + 1],
                                 scale=1.0 / (WSCALE * ESCALE))
            # fused (tanh * wv) + running-sum on DVE
            if dc == 0:
                wve_t = wve_pool.tile([P, HT], bf16, tag="wve",
                                      name=f"wve{b}_{h}_{dc}")
                nc.vector.tensor_scalar_mul(wve_t[:], t[:], wv_sb[:, 0:1])
                run_t[(b, h)] = wve_t
            else:
                nxt = wvs_pool.tile([P, HT], bf16, tag="wvs",
                                    name=f"wvs{b}_{h}_{dc}")
                nc.vector.scalar_tensor_tensor(
                    nxt[:], t[:], wv_sb[:, dc:dc + 1], run_t[(b, h)][:],
                    op0=ALU.mult, op1=ALU.add)
                run_t[(b, h)] = nxt

        def emit_scores(b, h):
            asum = run_t.pop((b, h))
            scth = ps_sc.tile([P, HSC], fp32, tag="sc", name=f"scth{b}_{h}")
            for sci in range(HSC):
                nc.tensor.matmul(scth[:, sci:sci + 1],
                                 lhsT=asum[:, sci * P:(sci + 1) * P],
                                 rhs=ones_sb[:], start=True, stop=True)
            # exp into probsT; accum_out = per-partition sums of this half
            nc.scalar.activation(probs_t[b][:, h * HSC:(h + 1) * HSC],
                                 scth[:, 0:HSC], AF.Exp,
                                 accum_out=acc_sb[:, NH * b + h:NH * b + h + 1])
            if h == NH - 1:
                gg, j = b // GRP, b % GRP
                nc.gpsimd.tensor_reduce(
                    ssum_sb[32 * j:32 * j + 1, gg:gg + 1],
                    acc_sb[:, NH * b:NH * b + 2],
                    axis=AX.XYZWC, op=ALU.add)
                nc.vector.reciprocal(rs_sb[32 * j:32 * j + 1, gg:gg + 1],
                                     ssum_sb[32 * j:32 * j + 1, gg:gg + 1])

        def emit_span(gg, h, c):
            # 4 concurrent col-tiled M=1 matmuls: batch gg*GRP+j on PE
            # column group j, output row at psum partition 32j
            if h == 0 and c == 0:
                orow_t[gg] = ps_or.tile([P, ENC], fp32, tag="or",
                                        name=f"orow{gg}")
            orow = orow_t[gg]
            for j in range(GRP):
                b = gg * GRP + j
                cc = h * HSC + c
                nc.tensor.matmul(
                    orow[32 * j:32 * j + 1, :],
                    lhsT=probs_t[b][:, cc:cc + 1],
                    rhs=encn_t[(b, h)][:, c, :],
                    tile_position=(0, 32 * j),
                    start=(cc == 0), stop=(cc == NSC - 1))

        def emit_finalize(gg):
            orow = orow_t.pop(gg)
            ostg = stage_pool.tile([P, ENC], fp32, tag="stg",
                                   name=f"ostg{gg}")
            nc.vector.tensor_scalar_mul(ostg[:], orow[:], rs_sb[:, gg:gg + 1])
            for j in range(GRP):
                b = gg * GRP + j
                nc.scalar.dma_start(out_d[b:b + 1, :],
                                    ostg[32 * j:32 * j + 1, :])

        def emit_e0(i):
            b, h = halves[i]
            if h == 0:
                probs_t[b] = probs_pool.tile([P, NSC], bf16, tag="probst",
                                             name=f"probsT{b}")
            issue_encn(b, h)
            emit_energy(b, h, 0)

        # pending weighted-sum work: list of closures, 2 drained per half
        pend = []

        def drain(n):
            for _ in range(min(n, len(pend))):
                pend.pop(0)()

        issue_enc4(*halves[0])
        emit_e0(0)
        for i, (b, h) in enumerate(halves):
            if i + 1 < len(halves):
                issue_enc4(*halves[i + 1])
            emit_energy(b, h, 1)
            emit_energy(b, h, 2)
            drain(1)
            emit_energy(b, h, 3)
            drain(1)
            if i + 1 < len(halves):
                emit_e0(i + 1)
            emit_scores(b, h)
            if i % GRP == GRP - 1:
                # sub-round (gg, h) complete: queue its 8 spans (+finalize)
                gg = b // GRP
                for c in range(HSC):
                    pend.append(lambda gg=gg, h=h, c=c: emit_span(gg, h, c))
                if h == NH - 1:
                    pend.append(lambda gg=gg: emit_finalize(gg))

        # keep the PE clock-gate warm through the final exp wait, then
        # drain the last sub-round's weighted sum + finalize
        wps2 = ps_e.tile([P, P], fp32, tag="pse", name="warmps2")
        for i in range(16):
            nc.tensor.matmul(wps2[:], lhsT=warm_sb[:], rhs=warm_sb[:],
                             start=(i == 0), stop=(i == 15))
        nc.vector.tensor_copy(warmout_sb[:], wps2[0:1, 0:1])
        drain(len(pend))

    nc.compile()
    return nc


def _get_program():
    global _PROGRAM
    if _PROGRAM is None:
        _PROGRAM = _build_program()
    return _PROGRAM


def _make_in_maps(hidden, encoder_outputs, W_attn, b_attn, w_v):
    import ml_dtypes
    bf = ml_dtypes.bfloat16
    f8 = ml_dtypes.float8_e4m3fn
    W_h, W_e = W_attn[:DEC], W_attn[DEC:]
    # [G, KO, P, DEC]: contraction index e = g*256 + ko*128 + ki
    # scaled x8 to keep fp8 mantissas in the normal range
    weT = np.ascontiguousarray(
        (np.asarray(W_e) * WSCALE).reshape(G, KO, P, DEC).astype(f8))
    wv = np.ascontiguousarray(np.asarray(w_v, np.float32).reshape(DC, P).T)
    # h_proj host-side: [B, DEC]
    hproj = (np.asarray(hidden, np.float32) @ np.asarray(W_h, np.float32)
             + np.asarray(b_attn, np.float32))
    in_maps = []
    for c in range(NCORES):
        eb = np.asarray(encoder_outputs[c * BL:(c + 1) * BL])
        # [BL, NST, P, G, KO, ST]: e = g*256 + ko*128 + p, one
        # contiguous 2KB row per partition per block; scaled x4
        enc4 = np.ascontiguousarray(
            (eb * ESCALE).transpose(0, 2, 1).reshape(BL, G, KO, P, NST, ST)
            .transpose(0, 4, 3, 1, 2, 5).astype(f8))
        # [BL, NH, P, HSC, ENC]: partition p gathers s = h*HT + c*P + p
        encN = np.ascontiguousarray(
            eb.reshape(BL, NH, HSC, P, ENC).transpose(0, 1, 3, 2, 4)
            .astype(bf))
        hp = hproj[c * BL:(c + 1) * BL]          # [BL, DEC]
        biasT = np.ascontiguousarray(
            hp.T.reshape(DC, P, BL).transpose(1, 0, 2))  # [P, DC, BL]
        in_maps.append({"encT4": enc4, "encN": encN, "weT": weT,
                        "biasT": biasT, "wv": wv})
    return in_maps


def _install_trace_hooks():
    """The agent image's antenv lacks axon_hooks; recreate it from the
    ctypes NTFF profile shim in trn_agent_boot, and stub the fish-bucket
    artifact upload so the trace path stays local."""
    import sys, types
    if "antenv.axon_hooks" not in sys.modules:
        mod = types.ModuleType("antenv.axon_hooks")
        mod._hook = None
        mod.set_axon_ntff_profile_hook = lambda h: setattr(mod, "_hook", h)
        mod.get_axon_ntff_profile_hook = lambda: mod._hook
        sys.modules["antenv.axon_hooks"] = mod
        import antenv
        antenv.axon_hooks = mod
        try:
            from trn_agent_boot.trn_boot import _ntff_profile_via_ctypes
            mod._hook = _ntff_profile_via_ctypes("/opt/axon/libaxon_pjrt.so")
        except Exception as e:
            print(f"NTFF hook install failed: {e}")
    import concourse.bass_utils as bu
    bu.upload_artifacts = lambda tmpdir: f"local:{tmpdir}"


def run(hidden, encoder_outputs, W_attn, b_attn, w_v, trace=False, tmpdir=None):
    from concourse.bass_utils import run_bass_kernel_spmd
    if trace:
        _install_trace_hooks()
    nc = _get_program()
    in_maps = _make_in_maps(hidden, encoder_outputs, W_attn, b_attn, w_v)
    res = run_bass_kernel_spmd(nc, in_maps, list(range(NCORES)),
                               trace=trace, tmpdir=tmpdir)
    out = np.concatenate([np.asarray(res.results[c]["out"], np.float32)
                          for c in range(NCORES)], axis=0)
    return out, res


def kernel(hidden, encoder_outputs, W_attn, b_attn, w_v):
    out, _ = run(hidden, encoder_outputs, W_attn, b_attn, w_v)
    return out


# revision 5
# speedup vs baseline: 1.1851x; 1.1851x over previous
"""Bahdanau-attention Trainium2 kernel (data-parallel over 8 NeuronCores).

Computation (per batch row b):
    energy[s, d] = tanh(hidden[b] @ W_h + enc[b, s] @ W_e + b_attn)   [S, D]
    scores[s]    = energy[s] . w_v                                     [S]
    attn         = softmax(scores)                                     [S]
    out[b]       = sum_s attn[s] * enc[b, s]                           [E]

v11 — batch-group col-tiled weighted sum, fused DVE chain, ACT-side
softmax sums:
  - halves processed in h-major sub-rounds of GRP=4 batches; the
    weighted sum for a sub-round runs as 8 spans of 4 col-tiled
    (tile_position=(0,32j)) M=1 matmuls — 4 batches stream
    concurrently on disjoint PE column groups, each batch's output row
    accumulating at psum partition 32j of one shared orow bank.
  - energy matmuls in fp8e4m3 DoubleRow (unchanged v10 structure);
    host stages enc*4 and W_e*8 to lift fp8 denormals, compensated
    with ACT scale=1/32 on the tanh.
  - wv multiply + running sum fused into one DVE scalar_tensor_tensor
    per dc chunk (was mul+add).
  - softmax denominator: exp accum_out gives per-partition sums free
    on ACT; GPSIMD XYZWC-reduce collapses them to [1,1] at partition
    32j; DVE reciprocal in place; one tensor_scalar scale per group.
  - psum: 3x2 banks energy + 1 orow + 1 scth = 8.
  - per-half emission: E1 E2 Wspan E3 Wspan E0' S exp, bursts spread
    2 spans per half across the following sub-round.
"""

import numpy as np

B, S, ENC, DEC = 64, 2048, 512, 512
NCORES = 8
BL = B // NCORES          # batches per core
P = 128
EC = ENC // P             # 4 e-chunks
DC = DEC // P             # 4 d-chunks
ST = 512                  # matmul moving free-dim tile / DMA block
NST = S // ST             # 4 s-blocks per batch
HT = 1024                 # psum energy tile free size (one half)
NH = S // HT              # 2 halves
NSC = S // P              # 16 s-chunks for the weighted sum
HSC = HT // P             # 8 s-chunks per half
NWARM = 64                # prewarm matmuls
G = 2                     # 256-wide DoubleRow contraction chunks
KO = 2                    # k-tiles per DoubleRow matmul
GRP = 4                   # batches per weighted-sum col-tile group
NGRP = BL // GRP
WSCALE = 8.0              # host-side W_e scale (fp8 denormal lift)
ESCALE = 4.0              # host-side enc scale for the energy copy

_PROGRAM = None


def _build_program():
    import concourse.mybir as mybir
    import concourse.tile as tile
    from concourse import bacc
    from contextlib import ExitStack

    fp32 = mybir.dt.float32
    bf16 = mybir.dt.bfloat16
    fp8 = mybir.dt.float8e4
    AF = mybir.ActivationFunctionType
    ALU = mybir.AluOpType
    AX = mybir.AxisListType

    nc = bacc.Bacc("TRN2", debug=False, target_bir_lowering=False,
                   num_devices=NCORES)

    enc4_d = nc.dram_tensor("encT4", [BL, NST, P, G, KO, ST], fp8,
                            kind="ExternalInput").ap()
    we_d = nc.dram_tensor("weT", [G, KO, P, DEC], fp8,
                          kind="ExternalInput").ap()
    encn_d = nc.dram_tensor("encN", [BL, NH, P, HSC, ENC], bf16,
                            kind="ExternalInput").ap()
    biasT_d = nc.dram_tensor("biasT", [P, DC, BL], fp32,
                             kind="ExternalInput").ap()
    wv_d = nc.dram_tensor("wv", [P, DC], fp32, kind="ExternalInput").ap()
    out_d = nc.dram_tensor("out", [BL, ENC], fp32, kind="ExternalOutput").ap()

    with tile.TileContext(nc) as tc, ExitStack() as ctx:
        const = ctx.enter_context(tc.tile_pool(name="const", bufs=1))
        # 3 energy-psum bufs (6 banks) + 1 orow bank + 1 scth bank = 8
        ps_e = ctx.enter_context(tc.tile_pool(name="ps_e", bufs=3, space="PSUM"))
        ps_or = ctx.enter_context(tc.tile_pool(name="ps_or", bufs=1, space="PSUM"))
        ps_sc = ctx.enter_context(tc.tile_pool(name="ps_sc", bufs=1, space="PSUM"))
        enc4_pool = ctx.enter_context(tc.tile_pool(name="enc4p", bufs=8))
        encn_pool = ctx.enter_context(tc.tile_pool(name="encnp", bufs=8))
        tanh_pool = ctx.enter_context(tc.tile_pool(name="tanhp", bufs=6))
        wve_pool = ctx.enter_context(tc.tile_pool(name="wvep", bufs=6))
        wvs_pool = ctx.enter_context(tc.tile_pool(name="wvsp", bufs=8))
        probs_pool = ctx.enter_context(tc.tile_pool(name="probsp", bufs=8))
        stage_pool = ctx.enter_context(tc.tile_pool(name="stagep", bufs=2))

        we_sb = const.tile([P, G, KO, DEC], fp8)
        biasT_sb = const.tile([P, DC, BL], fp32)
        wv_sb = const.tile([P, DC], fp32)
        ones_sb = const.tile([P, 1], bf16)
        warm_sb = const.tile([P, P], bf16)
        warmout_sb = const.tile([1, 1], fp32)
        ssum_sb = const.tile([P, NGRP], fp32)      # denominators at 32j
        rs_sb = const.tile([P, NGRP], fp32)        # reciprocals at 32j
        iscale_sb = const.tile([P, 1], fp32)       # 1/(WSCALE*ESCALE)

        nc.vector.memset(warm_sb[:], 1.0)
        nc.vector.memset(ones_sb[:], 1.0)
        nc.vector.memset(rs_sb[:], 1.0)
        nc.vector.memset(ssum_sb[:], 1.0)
        nc.vector.memset(iscale_sb[:], 1.0 / (WSCALE * ESCALE))

        nc.sync.dma_start(we_sb[:], we_d.rearrange("g k p d -> p g k d"))
        nc.scalar.dma_start(biasT_sb[:], biasT_d)
        nc.scalar.dma_start(wv_sb[:], wv_d)

        # HAM prewarm: dummy accumulating matmuls, no data deps
        wps = ps_e.tile([P, P], fp32, tag="pse", name="warmps")
        for i in range(NWARM):
            nc.tensor.matmul(wps[:], lhsT=warm_sb[:], rhs=warm_sb[:],
                             start=(i == 0), stop=(i == NWARM - 1))
        nc.vector.tensor_copy(warmout_sb[:], wps[0:1, 0:1])

        enc4_t, encn_t = {}, {}
        probs_t, orow_t = {}, {}
        run_t = {}

        halves = [(gg * GRP + j, h) for gg in range(NGRP)
                  for h in range(NH) for j in range(GRP)]

        def issue_enc4(b, h):
            for st in (2 * h, 2 * h + 1):
                t = enc4_pool.tile([P, G, KO, ST], fp8, tag="enc4",
                                   name=f"enc4_{b}_{st}")
                eng = nc.scalar if st % 2 == 1 else nc.sync
                eng.dma_start(t[:], enc4_d[b, st])
                enc4_t[(b, st)] = t

        def issue_encn(b, h):
            t = encn_pool.tile([P, HSC, ENC], bf16, tag="encn",
                               name=f"encn{b}_{h}")
            nc.sync.dma_start(t[:], encn_d[b, h])
            encn_t[(b, h)] = t

        def emit_energy(b, h, dc):
            eps = ps_e.tile([P, HT], fp32, tag="pse", name=f"eps{b}_{h}_{dc}")
            for st in range(HT // ST):
                blk = h * (HT // ST) + st
                for g in range(G):
                    nc.tensor.matmul(
                        eps[:, st * ST:(st + 1) * ST],
                        lhsT=we_sb[:, g, :, dc * P:(dc + 1) * P],
                        rhs=enc4_t[(b, blk)][:, g, :, :],
                        start=(g == 0), stop=(g == G - 1),
                        perf_mode=mybir.MatmulPerfMode.DoubleRow)
            t = tanh_pool.tile([P, HT], bf16, tag="tanh",
                               name=f"tanh{b}_{h}_{dc}")
            nc.scalar.activation(t[:], eps[:], AF.Tanh,
                                 bias=biasT_sb[:, dc, b:b + 1],
                                 scale=iscale_sb[:, 0:1])
            # wv multiply + running sum on DVE
            wve_t = wve_pool.tile([P, HT], bf16, tag="wve",
                                  name=f"wve{b}_{h}_{dc}")
            nc.vector.tensor_scalar_mul(wve_t[:], t[:], wv_sb[:, dc:dc + 1])
            if dc == 0:
                run_t[(b, h)] = wve_t
            else:
                nxt = wvs_pool.tile([P, HT], bf16, tag="wvs",
                                    name=f"wvs{b}_{h}_{dc}")
                nc.vector.tensor_add(nxt[:], run_t[(b, h)][:], wve_t[:])
                run_t[(b, h)] = nxt

        def emit_scores(b, h):
            asum = run_t.pop((b, h))
            scth = ps_sc.tile([P, NSC], fp32, tag="sc", name=f"scth{b}_{h}")
            for sci in range(HSC):
                nc.tensor.matmul(scth[:, sci:sci + 1],
                                 lhsT=asum[:, sci * P:(sci + 1) * P],
                                 rhs=ones_sb[:], start=True, stop=True)
            nc.scalar.activation(probs_t[b][:, h * HSC:(h + 1) * HSC],
                                 scth[:, 0:HSC], AF.Exp)
            if h == NH - 1:
                # softmax denominator at partition 32j: col-tiled ones
                # matmul into the retired scth row, reduce+recip in lane
                gg, j = b // GRP, b % GRP
                nc.tensor.matmul(scth[32 * j:32 * j + 1, 0:NSC],
                                 lhsT=ones_sb[:], rhs=probs_t[b][:, 0:NSC],
                                 tile_position=(0, 32 * j),
                                 start=True, stop=True)
                nc.vector.tensor_reduce(ssum_sb[32 * j:32 * j + 1, gg:gg + 1],
                                        scth[32 * j:32 * j + 1, 0:NSC],
                                        axis=AX.X, op=ALU.add)
                nc.vector.reciprocal(rs_sb[32 * j:32 * j + 1, gg:gg + 1],
                                     ssum_sb[32 * j:32 * j + 1, gg:gg + 1])

        def emit_span(gg, h, c):
            # 4 concurrent col-tiled M=1 matmuls: batch gg*GRP+j on PE
            # column group j, output row at psum partition 32j
            if h == 0 and c == 0:
                orow_t[gg] = ps_or.tile([P, ENC], fp32, tag="or",
                                        name=f"orow{gg}")
            orow = orow_t[gg]
            for j in range(GRP):
                b = gg * GRP + j
                cc = h * HSC + c
                nc.tensor.matmul(
                    orow[32 * j:32 * j + 1, :],
                    lhsT=probs_t[b][:, cc:cc + 1],
                    rhs=encn_t[(b, h)][:, c, :],
                    tile_position=(0, 32 * j),
                    start=(cc == 0), stop=(cc == NSC - 1))

        def emit_finalize(gg):
            orow = orow_t.pop(gg)
            ostg = stage_pool.tile([P, ENC], fp32, tag="stg",
                                   name=f"ostg{gg}")
            nc.vector.tensor_scalar_mul(ostg[:], orow[:], rs_sb[:, gg:gg + 1])
            for j in range(GRP):
                b = gg * GRP + j
                nc.scalar.dma_start(out_d[b:b + 1, :],
                                    ostg[32 * j:32 * j + 1, :])

        def emit_e0(i):
            b, h = halves[i]
            if h == 0:
                probs_t[b] = probs_pool.tile([P, NSC], bf16, tag="probst",
                                             name=f"probsT{b}")
            issue_encn(b, h)
            emit_energy(b, h, 0)

        # pending weighted-sum work: list of closures, 2 drained per half
        pend = []

        def drain(n):
            for _ in range(min(n, len(pend))):
                pend.pop(0)()

        issue_enc4(*halves[0])
        emit_e0(0)
        for i, (b, h) in enumerate(halves):
            if i + 1 < len(halves):
                issue_enc4(*halves[i + 1])
            emit_energy(b, h, 1)
            emit_energy(b, h, 2)
            drain(1)
            emit_energy(b, h, 3)
            drain(1)
            if i + 1 < len(halves):
                emit_e0(i + 1)
            emit_scores(b, h)
            if i % GRP == GRP - 1:
                # sub-round (gg, h) complete: queue its 8 spans (+finalize)
                gg = b // GRP
                for c in range(HSC):
                    pend.append(lambda gg=gg, h=h, c=c: emit_span(gg, h, c))
                if h == NH - 1:
                    pend.append(lambda gg=gg: emit_finalize(gg))

        # keep the PE clock-gate warm through the final exp wait, then
        # drain the last sub-round's weighted sum + finalize
        wps2 = ps_e.tile([P, P], fp32, tag="pse", name="warmps2")
        for i in range(16):
            nc.tensor.matmul(wps2[:], lhsT=warm_sb[:], rhs=warm_sb[:],
                             start=(i == 0), stop=(i == 15))
        nc.vector.tensor_copy(warmout_sb[:], wps2[0:1, 0:1])
        drain(len(pend))

    nc.compile()
    return nc


def _get_program():
    global _PROGRAM
    if _PROGRAM is None:
        _PROGRAM = _build_program()
    return _PROGRAM


def _make_in_maps(hidden, encoder_outputs, W_attn, b_attn, w_v):
    import ml_dtypes
    bf = ml_dtypes.bfloat16
    f8 = ml_dtypes.float8_e4m3fn
    W_h, W_e = W_attn[:DEC], W_attn[DEC:]
    # [G, KO, P, DEC]: contraction index e = g*256 + ko*128 + ki
    # scaled x8 to keep fp8 mantissas in the normal range
    weT = np.ascontiguousarray(
        (np.asarray(W_e) * WSCALE).reshape(G, KO, P, DEC).astype(f8))
    wv = np.ascontiguousarray(np.asarray(w_v, np.float32).reshape(DC, P).T)
    # h_proj host-side: [B, DEC]
    hproj = (np.asarray(hidden, np.float32) @ np.asarray(W_h, np.float32)
             + np.asarray(b_attn, np.float32))
    in_maps = []
    for c in range(NCORES):
        eb = np.asarray(encoder_outputs[c * BL:(c + 1) * BL])
        # [BL, NST, P, G, KO, ST]: e = g*256 + ko*128 + p, one
        # contiguous 2KB row per partition per block; scaled x4
        enc4 = np.ascontiguousarray(
            (eb * ESCALE).transpose(0, 2, 1).reshape(BL, G, KO, P, NST, ST)
            .transpose(0, 4, 3, 1, 2, 5).astype(f8))
        # [BL, NH, P, HSC, ENC]: partition p gathers s = h*HT + c*P + p
        encN = np.ascontiguousarray(
            eb.reshape(BL, NH, HSC, P, ENC).transpose(0, 1, 3, 2, 4)
            .astype(bf))
        hp = hproj[c * BL:(c + 1) * BL]          # [BL, DEC]
        biasT = np.ascontiguousarray(
            hp.T.reshape(DC, P, BL).transpose(1, 0, 2))  # [P, DC, BL]
        in_maps.append({"encT4": enc4, "encN": encN, "weT": weT,
                        "biasT": biasT, "wv": wv})
    return in_maps


def _install_trace_hooks():
    """The agent image's antenv lacks axon_hooks; recreate it from the
    ctypes NTFF profile shim in trn_agent_boot, and stub the fish-bucket
    artifact upload so the trace path stays local."""
    import sys, types
    if "antenv.axon_hooks" not in sys.modules:
        mod = types.ModuleType("antenv.axon_hooks")
        mod._hook = None
        mod.set_axon_ntff_profile_hook = lambda h: setattr(mod, "_hook", h)
        mod.get_axon_ntff_profile_hook = lambda: mod._hook
        sys.modules["antenv.axon_hooks"] = mod
        import antenv
        antenv.axon_hooks = mod
        try:
            from trn_agent_boot.trn_boot import _ntff_profile_via_ctypes
            mod._hook = _ntff_profile_via_ctypes("/opt/axon/libaxon_pjrt.so")
        except Exception as e:
            print(f"NTFF hook install failed: {e}")
    import concourse.bass_utils as bu
    bu.upload_artifacts = lambda tmpdir: f"local:{tmpdir}"


def run(hidden, encoder_outputs, W_attn, b_attn, w_v, trace=False, tmpdir=None):
    from concourse.bass_utils import run_bass_kernel_spmd
    if trace:
        _install_trace_hooks()
    nc = _get_program()
    in_maps = _make_in_maps(hidden, encoder_outputs, W_attn, b_attn, w_v)
    res = run_bass_kernel_spmd(nc, in_maps, list(range(NCORES)),
                               trace=trace, tmpdir=tmpdir)
    out = np.concatenate([np.asarray(res.results[c]["out"], np.float32)
                          for c in range(NCORES)], axis=0)
    return out, res


def kernel(hidden, encoder_outputs, W_attn, b_attn, w_v):
    out, _ = run(hidden, encoder_outputs, W_attn, b_attn, w_v)
    return out


# revision 8
# speedup vs baseline: 1.4746x; 1.2442x over previous
"""Bahdanau-attention Trainium2 kernel (data-parallel over 8 NeuronCores).

Computation (per batch row b):
    energy[s, d] = tanh(hidden[b] @ W_h + enc[b, s] @ W_e + b_attn)   [S, D]
    scores[s]    = energy[s] . w_v                                     [S]
    attn         = softmax(scores)                                     [S]
    out[b]       = sum_s attn[s] * enc[b, s]                           [E]

v11 — batch-group col-tiled weighted sum, fused DVE chain, ACT-side
softmax sums:
  - halves processed in h-major sub-rounds of GRP=4 batches; the
    weighted sum for a sub-round runs as 8 spans of 4 col-tiled
    (tile_position=(0,32j)) M=1 matmuls — 4 batches stream
    concurrently on disjoint PE column groups, each batch's output row
    accumulating at psum partition 32j of one shared orow bank.
  - energy matmuls in fp8e4m3 DoubleRow (unchanged v10 structure);
    host stages enc*4 and W_e*8 to lift fp8 denormals, compensated
    with ACT scale=1/32 on the tanh.
  - wv multiply + running sum fused into one DVE scalar_tensor_tensor
    per dc chunk (was mul+add).
  - softmax denominator: exp accum_out gives per-partition sums free
    on ACT; GPSIMD XYZWC-reduce collapses them to [1,1] at partition
    32j; DVE reciprocal in place; one tensor_scalar scale per group.
  - psum: 3x2 banks energy + 1 orow + 1 scth = 8.
  - per-half emission: E1 E2 Wspan E3 Wspan E0' S exp, bursts spread
    2 spans per half across the following sub-round.
"""

import numpy as np

B, S, ENC, DEC = 64, 2048, 512, 512
NCORES = 8
BL = B // NCORES          # batches per core
P = 128
EC = ENC // P             # 4 e-chunks
DC = DEC // P             # 4 d-chunks
ST = 512                  # matmul moving free-dim tile / DMA block
NST = S // ST             # 4 s-blocks per batch
HT = 1024                 # psum energy tile free size (one half)
NH = S // HT              # 2 halves
NSC = S // P              # 16 s-chunks for the weighted sum
HSC = HT // P             # 8 s-chunks per half
NWARM = 64                # prewarm matmuls
G = 2                     # 256-wide DoubleRow contraction chunks
KO = 2                    # k-tiles per DoubleRow matmul
GRP = 4                   # batches per weighted-sum col-tile group
NGRP = BL // GRP
WSCALE = 8.0              # host-side W_e scale (fp8 denormal lift)
ESCALE = 4.0              # host-side enc scale for the energy copy

_PROGRAM = None


def _build_program():
    import concourse.mybir as mybir
    import concourse.tile as tile
    from concourse import bacc
    from contextlib import ExitStack

    fp32 = mybir.dt.float32
    bf16 = mybir.dt.bfloat16
    fp8 = mybir.dt.float8e4
    AF = mybir.ActivationFunctionType
    ALU = mybir.AluOpType
    AX = mybir.AxisListType

    nc = bacc.Bacc("TRN2", debug=False, target_bir_lowering=False,
                   num_devices=NCORES)

    enc4_d = nc.dram_tensor("encT4", [BL, NST, P, G, KO, ST], fp8,
                            kind="ExternalInput").ap()
    we_d = nc.dram_tensor("weT", [G, KO, P, DEC], fp8,
                          kind="ExternalInput").ap()
    encn_d = nc.dram_tensor("encN", [BL, NH, P, HSC, ENC], bf16,
                            kind="ExternalInput").ap()
    biasT_d = nc.dram_tensor("biasT", [P, DC, BL], fp32,
                             kind="ExternalInput").ap()
    wv_d = nc.dram_tensor("wv", [P, DC], fp32, kind="ExternalInput").ap()
    out_d = nc.dram_tensor("out", [BL, ENC], fp32, kind="ExternalOutput").ap()

    with tile.TileContext(nc) as tc, ExitStack() as ctx:
        const = ctx.enter_context(tc.tile_pool(name="const", bufs=1))
        # 3 energy-psum bufs (6 banks) + 1 orow bank + 1 scth bank = 8
        ps_e = ctx.enter_context(tc.tile_pool(name="ps_e", bufs=3, space="PSUM"))
        ps_or = ctx.enter_context(tc.tile_pool(name="ps_or", bufs=1, space="PSUM"))
        ps_sc = ctx.enter_context(tc.tile_pool(name="ps_sc", bufs=1, space="PSUM"))
        enc4_pool = ctx.enter_context(tc.tile_pool(name="enc4p", bufs=8))
        encn_pool = ctx.enter_context(tc.tile_pool(name="encnp", bufs=8))
        tanh_pool = ctx.enter_context(tc.tile_pool(name="tanhp", bufs=6))
        wve_pool = ctx.enter_context(tc.tile_pool(name="wvep", bufs=6))
        wvs_pool = ctx.enter_context(tc.tile_pool(name="wvsp", bufs=8))
        probs_pool = ctx.enter_context(tc.tile_pool(name="probsp", bufs=8))
        stage_pool = ctx.enter_context(tc.tile_pool(name="stagep", bufs=2))

        we_sb = const.tile([P, G, KO, DEC], fp8)
        biasT_sb = const.tile([P, DC, BL], fp32)
        wv_sb = const.tile([P, DC], fp32)
        ones_sb = const.tile([P, 1], bf16)
        warm_sb = const.tile([P, P], bf16)
        warmout_sb = const.tile([1, 1], fp32)
        ssum_sb = const.tile([P, NGRP], fp32)      # denominators at 32j
        rs_sb = const.tile([P, NGRP], fp32)        # reciprocals at 32j
        iscale_sb = const.tile([P, 1], fp32)       # 1/(WSCALE*ESCALE)

        nc.vector.memset(warm_sb[:], 1.0)
        nc.vector.memset(ones_sb[:], 1.0)
        nc.vector.memset(rs_sb[:], 1.0)
        nc.vector.memset(ssum_sb[:], 1.0)
        nc.vector.memset(iscale_sb[:], 1.0 / (WSCALE * ESCALE))

        nc.sync.dma_start(we_sb[:], we_d.rearrange("g k p d -> p g k d"))
        nc.scalar.dma_start(biasT_sb[:], biasT_d)
        nc.scalar.dma_start(wv_sb[:], wv_d)

        # HAM prewarm: dummy accumulating matmuls, no data deps
        wps = ps_e.tile([P, P], fp32, tag="pse", name="warmps")
        for i in range(NWARM):
            nc.tensor.matmul(wps[:], lhsT=warm_sb[:], rhs=warm_sb[:],
                             start=(i == 0), stop=(i == NWARM - 1))
        nc.vector.tensor_copy(warmout_sb[:], wps[0:1, 0:1])

        enc4_t, encn_t = {}, {}
        probs_t, orow_t = {}, {}
        run_t = {}

        halves = [(gg * GRP + j, h) for gg in range(NGRP)
                  for h in range(NH) for j in range(GRP)]

        def issue_enc4(b, h):
            for st in (2 * h, 2 * h + 1):
                t = enc4_pool.tile([P, G, KO, ST], fp8, tag="enc4",
                                   name=f"enc4_{b}_{st}")
                # batch 0: odd block via the scalar HWDGE ring so both
                # FIFO chains deliver the first half in parallel
                eng = nc.scalar if (b == 0 and h == 0 and st == 1) else nc.sync
                eng.dma_start(t[:], enc4_d[b, st])
                enc4_t[(b, st)] = t

        def issue_encn(b, h):
            t = encn_pool.tile([P, HSC, ENC], bf16, tag="encn",
                               name=f"encn{b}_{h}")
            nc.sync.dma_start(t[:], encn_d[b, h])
            encn_t[(b, h)] = t

        def emit_energy(b, h, dc):
            eps = ps_e.tile([P, HT], fp32, tag="pse", name=f"eps{b}_{h}_{dc}")
            for st in range(HT // ST):
                blk = h * (HT // ST) + st
                for g in range(G):
                    nc.tensor.matmul(
                        eps[:, st * ST:(st + 1) * ST],
                        lhsT=we_sb[:, g, :, dc * P:(dc + 1) * P],
                        rhs=enc4_t[(b, blk)][:, g, :, :],
                        start=(g == 0), stop=(g == G - 1),
                        perf_mode=mybir.MatmulPerfMode.DoubleRow)
            t = tanh_pool.tile([P, HT], bf16, tag="tanh",
                               name=f"tanh{b}_{h}_{dc}")
            nc.scalar.activation(t[:], eps[:], AF.Tanh,
                                 bias=biasT_sb[:, dc, b:b + 1],
                                 scale=iscale_sb[:, 0:1])
            # wv multiply + running sum on DVE
            wve_t = wve_pool.tile([P, HT], bf16, tag="wve",
                                  name=f"wve{b}_{h}_{dc}")
            nc.vector.tensor_scalar_mul(wve_t[:], t[:], wv_sb[:, dc:dc + 1])
            if dc == 0:
                run_t[(b, h)] = wve_t
            else:
                nxt = wvs_pool.tile([P, HT], bf16, tag="wvs",
                                    name=f"wvs{b}_{h}_{dc}")
                nc.vector.tensor_add(nxt[:], run_t[(b, h)][:], wve_t[:])
                run_t[(b, h)] = nxt

        def emit_scores(b, h):
            asum = run_t.pop((b, h))
            scth = ps_sc.tile([P, NSC], fp32, tag="sc", name=f"scth{b}_{h}")
            for sci in range(HSC):
                nc.tensor.matmul(scth[:, sci:sci + 1],
                                 lhsT=asum[:, sci * P:(sci + 1) * P],
                                 rhs=ones_sb[:], start=True, stop=True)
            nc.scalar.activation(probs_t[b][:, h * HSC:(h + 1) * HSC],
                                 scth[:, 0:HSC], AF.Exp)
            if h == NH - 1:
                # softmax denominator at partition 32j: col-tiled ones
                # matmul into the retired scth row, reduce+recip in lane
                gg, j = b // GRP, b % GRP
                nc.tensor.matmul(scth[32 * j:32 * j + 1, 0:NSC],
                                 lhsT=ones_sb[:], rhs=probs_t[b][:, 0:NSC],
                                 tile_position=(0, 32 * j),
                                 start=True, stop=True)
                nc.vector.tensor_reduce(ssum_sb[32 * j:32 * j + 1, gg:gg + 1],
                                        scth[32 * j:32 * j + 1, 0:NSC],
                                        axis=AX.X, op=ALU.add)
                nc.vector.reciprocal(rs_sb[32 * j:32 * j + 1, gg:gg + 1],
                                     ssum_sb[32 * j:32 * j + 1, gg:gg + 1])

        def emit_span(gg, h, c):
            # 4 concurrent col-tiled M=1 matmuls: batch gg*GRP+j on PE
            # column group j, output row at psum partition 32j
            if h == 0 and c == 0:
                orow_t[gg] = ps_or.tile([P, ENC], fp32, tag="or",
                                        name=f"orow{gg}")
            orow = orow_t[gg]
            for j in range(GRP):
                b = gg * GRP + j
                cc = h * HSC + c
                nc.tensor.matmul(
                    orow[32 * j:32 * j + 1, :],
                    lhsT=probs_t[b][:, cc:cc + 1],
                    rhs=encn_t[(b, h)][:, c, :],
                    tile_position=(0, 32 * j),
                    start=(cc == 0), stop=(cc == NSC - 1))

        def emit_finalize(gg):
            orow = orow_t.pop(gg)
            ostg = stage_pool.tile([P, ENC], fp32, tag="stg",
                                   name=f"ostg{gg}")
            nc.vector.tensor_scalar_mul(ostg[:], orow[:], rs_sb[:, gg:gg + 1])
            for j in range(GRP):
                b = gg * GRP + j
                nc.sync.dma_start(out_d[b:b + 1, :],
                                  ostg[32 * j:32 * j + 1, :])

        def emit_e0(i):
            b, h = halves[i]
            if h == 0:
                probs_t[b] = probs_pool.tile([P, NSC], bf16, tag="probst",
                                             name=f"probsT{b}")
            issue_encn(b, h)
            emit_energy(b, h, 0)

        # pending weighted-sum work: list of closures, 2 drained per half
        pend = []

        def drain(n):
            for _ in range(min(n, len(pend))):
                pend.pop(0)()

        issue_enc4(*halves[0])
        issue_enc4(*halves[1])
        emit_e0(0)
        for i, (b, h) in enumerate(halves):
            if i + 2 < len(halves):
                issue_enc4(*halves[i + 2])
            emit_energy(b, h, 1)
            emit_energy(b, h, 2)
            drain(1)
            emit_energy(b, h, 3)
            drain(1)
            if i + 1 < len(halves):
                emit_e0(i + 1)
            emit_scores(b, h)
            if i % GRP == GRP - 1:
                # sub-round (gg, h) complete: queue its 8 spans (+finalize)
                gg = b // GRP
                for c in range(HSC):
                    pend.append(lambda gg=gg, h=h, c=c: emit_span(gg, h, c))
                if h == NH - 1:
                    pend.append(lambda gg=gg: emit_finalize(gg))

        # keep the PE clock-gate warm through the final exp wait, then
        # drain the last sub-round's weighted sum + finalize
        wps2 = ps_e.tile([P, P], fp32, tag="pse", name="warmps2")
        for i in range(16):
            nc.tensor.matmul(wps2[:], lhsT=warm_sb[:], rhs=warm_sb[:],
                             start=(i == 0), stop=(i == 15))
        nc.vector.tensor_copy(warmout_sb[:], wps2[0:1, 0:1])
        drain(len(pend))

    nc.compile()
    return nc


def _get_program():
    global _PROGRAM
    if _PROGRAM is None:
        _PROGRAM = _build_program()
    return _PROGRAM


def _make_in_maps(hidden, encoder_outputs, W_attn, b_attn, w_v):
    import ml_dtypes
    bf = ml_dtypes.bfloat16
    f8 = ml_dtypes.float8_e4m3fn
    W_h, W_e = W_attn[:DEC], W_attn[DEC:]
    # [G, KO, P, DEC]: contraction index e = g*256 + ko*128 + ki
    # scaled x8 to keep fp8 mantissas in the normal range
    weT = np.ascontiguousarray(
        (np.asarray(W_e) * WSCALE).reshape(G, KO, P, DEC).astype(f8))
    wv = np.ascontiguousarray(np.asarray(w_v, np.float32).reshape(DC, P).T)
    # h_proj host-side: [B, DEC]
    hproj = (np.asarray(hidden, np.float32) @ np.asarray(W_h, np.float32)
             + np.asarray(b_attn, np.float32))
    in_maps = []
    for c in range(NCORES):
        eb = np.asarray(encoder_outputs[c * BL:(c + 1) * BL])
        # [BL, NST, P, G, KO, ST]: e = g*256 + ko*128 + p, one
        # contiguous 2KB row per partition per block; scaled x4
        enc4 = np.ascontiguousarray(
            (eb * ESCALE).transpose(0, 2, 1).reshape(BL, G, KO, P, NST, ST)
            .transpose(0, 4, 3, 1, 2, 5).astype(f8))
        # [BL, NH, P, HSC, ENC]: partition p gathers s = h*HT + c*P + p
        encN = np.ascontiguousarray(
            eb.reshape(BL, NH, HSC, P, ENC).transpose(0, 1, 3, 2, 4)
            .astype(bf))
        hp = hproj[c * BL:(c + 1) * BL]          # [BL, DEC]
        biasT = np.ascontiguousarray(
            hp.T.reshape(DC, P, BL).transpose(1, 0, 2))  # [P, DC, BL]
        in_maps.append({"encT4": enc4, "encN": encN, "weT": weT,
                        "biasT": biasT, "wv": wv})
    return in_maps


def _install_trace_hooks():
    """The agent image's antenv lacks axon_hooks; recreate it from the
    ctypes NTFF profile shim in trn_agent_boot, and stub the fish-bucket
    artifact upload so the trace path stays local."""
    import sys, types
    if "antenv.axon_hooks" not in sys.modules:
        mod = types.ModuleType("antenv.axon_hooks")
        mod._hook = None
        mod.set_axon_ntff_profile_hook = lambda h: setattr(mod, "_hook", h)
        mod.get_axon_ntff_profile_hook = lambda: mod._hook
        sys.modules["antenv.axon_hooks"] = mod
        import antenv
        antenv.axon_hooks = mod
        try:
            from trn_agent_boot.trn_boot import _ntff_profile_via_ctypes
            mod._hook = _ntff_profile_via_ctypes("/opt/axon/libaxon_pjrt.so")
        except Exception as e:
            print(f"NTFF hook install failed: {e}")
    import concourse.bass_utils as bu
    bu.upload_artifacts = lambda tmpdir: f"local:{tmpdir}"


def run(hidden, encoder_outputs, W_attn, b_attn, w_v, trace=False, tmpdir=None):
    from concourse.bass_utils import run_bass_kernel_spmd
    if trace:
        _install_trace_hooks()
    nc = _get_program()
    in_maps = _make_in_maps(hidden, encoder_outputs, W_attn, b_attn, w_v)
    res = run_bass_kernel_spmd(nc, in_maps, list(range(NCORES)),
                               trace=trace, tmpdir=tmpdir)
    out = np.concatenate([np.asarray(res.results[c]["out"], np.float32)
                          for c in range(NCORES)], axis=0)
    return out, res


def kernel(hidden, encoder_outputs, W_attn, b_attn, w_v):
    out, _ = run(hidden, encoder_outputs, W_attn, b_attn, w_v)
    return out


# revision 10
# speedup vs baseline: 1.5129x; 1.0260x over previous
"""Bahdanau-attention Trainium2 kernel (data-parallel over 8 NeuronCores).

Computation (per batch row b):
    energy[s, d] = tanh(hidden[b] @ W_h + enc[b, s] @ W_e + b_attn)   [S, D]
    scores[s]    = energy[s] . w_v                                     [S]
    attn         = softmax(scores)                                     [S]
    out[b]       = sum_s attn[s] * enc[b, s]                           [E]

v11 — batch-group col-tiled weighted sum, fused DVE chain, ACT-side
softmax sums:
  - halves processed in h-major sub-rounds of GRP=4 batches; the
    weighted sum for a sub-round runs as 8 spans of 4 col-tiled
    (tile_position=(0,32j)) M=1 matmuls — 4 batches stream
    concurrently on disjoint PE column groups, each batch's output row
    accumulating at psum partition 32j of one shared orow bank.
  - energy matmuls in fp8e4m3 DoubleRow (unchanged v10 structure);
    host stages enc*4 and W_e*8 to lift fp8 denormals, compensated
    with ACT scale=1/32 on the tanh.
  - wv multiply + running sum fused into one DVE scalar_tensor_tensor
    per dc chunk (was mul+add).
  - softmax denominator: exp accum_out gives per-partition sums free
    on ACT; GPSIMD XYZWC-reduce collapses them to [1,1] at partition
    32j; DVE reciprocal in place; one tensor_scalar scale per group.
  - psum: 3x2 banks energy + 1 orow + 1 scth = 8.
  - per-half emission: E1 E2 Wspan E3 Wspan E0' S exp, bursts spread
    2 spans per half across the following sub-round.
"""

import numpy as np

B, S, ENC, DEC = 64, 2048, 512, 512
NCORES = 8
BL = B // NCORES          # batches per core
P = 128
EC = ENC // P             # 4 e-chunks
DC = DEC // P             # 4 d-chunks
ST = 512                  # matmul moving free-dim tile / DMA block
NST = S // ST             # 4 s-blocks per batch
HT = 1024                 # psum energy tile free size (one half)
NH = S // HT              # 2 halves
NSC = S // P              # 16 s-chunks for the weighted sum
HSC = HT // P             # 8 s-chunks per half
NWARM = 64                # prewarm matmuls
G = 2                     # 256-wide DoubleRow contraction chunks
KO = 2                    # k-tiles per DoubleRow matmul
GRP = 4                   # batches per weighted-sum col-tile group
NGRP = BL // GRP
WSCALE = 8.0              # host-side W_e scale (fp8 denormal lift)
ESCALE = 4.0              # host-side enc scale for the energy copy

_PROGRAM = None


def _build_program():
    import concourse.mybir as mybir
    import concourse.tile as tile
    from concourse import bacc
    from contextlib import ExitStack

    fp32 = mybir.dt.float32
    bf16 = mybir.dt.bfloat16
    fp8 = mybir.dt.float8e4
    AF = mybir.ActivationFunctionType
    ALU = mybir.AluOpType
    AX = mybir.AxisListType

    nc = bacc.Bacc("TRN2", debug=False, target_bir_lowering=False,
                   num_devices=NCORES)

    enc4_d = nc.dram_tensor("encT4", [BL, NST, P, G, KO, ST], fp8,
                            kind="ExternalInput").ap()
    we_d = nc.dram_tensor("weT", [G, KO, P, DEC], fp8,
                          kind="ExternalInput").ap()
    encn_d = nc.dram_tensor("encN", [BL, NH, P, HSC, ENC], bf16,
                            kind="ExternalInput").ap()
    biasT_d = nc.dram_tensor("biasT", [P, DC, BL], fp32,
                             kind="ExternalInput").ap()
    wv_d = nc.dram_tensor("wv", [P, DC], fp32, kind="ExternalInput").ap()
    out_d = nc.dram_tensor("out", [BL, ENC], fp32, kind="ExternalOutput").ap()

    with tile.TileContext(nc) as tc, ExitStack() as ctx:
        const = ctx.enter_context(tc.tile_pool(name="const", bufs=1))
        # 3 energy-psum bufs (6 banks) + 1 orow bank + 1 scth bank = 8
        ps_e = ctx.enter_context(tc.tile_pool(name="ps_e", bufs=3, space="PSUM"))
        ps_or = ctx.enter_context(tc.tile_pool(name="ps_or", bufs=1, space="PSUM"))
        ps_sc = ctx.enter_context(tc.tile_pool(name="ps_sc", bufs=1, space="PSUM"))
        enc4_pool = ctx.enter_context(tc.tile_pool(name="enc4p", bufs=8))
        encn_pool = ctx.enter_context(tc.tile_pool(name="encnp", bufs=8))
        tanh_pool = ctx.enter_context(tc.tile_pool(name="tanhp", bufs=6))
        wve_pool = ctx.enter_context(tc.tile_pool(name="wvep", bufs=6))
        wvs_pool = ctx.enter_context(tc.tile_pool(name="wvsp", bufs=8))
        probs_pool = ctx.enter_context(tc.tile_pool(name="probsp", bufs=8))
        stage_pool = ctx.enter_context(tc.tile_pool(name="stagep", bufs=2))

        we_sb = const.tile([P, G, KO, DEC], fp8)
        biasT_sb = const.tile([P, DC, BL], fp32)
        wv_sb = const.tile([P, DC], fp32)
        ones_sb = const.tile([P, 1], bf16)
        warm_sb = const.tile([P, P], bf16)
        warmout_sb = const.tile([1, 1], fp32)
        ssum_sb = const.tile([P, NGRP], fp32)      # denominators at 32j
        rs_sb = const.tile([P, NGRP], fp32)        # reciprocals at 32j
        iscale_sb = const.tile([P, 1], fp32)       # 1/(WSCALE*ESCALE)

        nc.vector.memset(warm_sb[:], 1.0)
        nc.vector.memset(ones_sb[:], 1.0)
        nc.vector.memset(rs_sb[:], 1.0)
        nc.vector.memset(ssum_sb[:], 1.0)
        nc.vector.memset(iscale_sb[:], 1.0 / (WSCALE * ESCALE))

        nc.sync.dma_start(we_sb[:], we_d.rearrange("g k p d -> p g k d"))
        nc.scalar.dma_start(biasT_sb[:], biasT_d)
        nc.scalar.dma_start(wv_sb[:], wv_d)

        # HAM prewarm: dummy accumulating matmuls, no data deps
        wps = ps_e.tile([P, P], fp32, tag="pse", name="warmps")
        for i in range(NWARM):
            nc.tensor.matmul(wps[:], lhsT=warm_sb[:], rhs=warm_sb[:],
                             start=(i == 0), stop=(i == NWARM - 1))
        nc.vector.tensor_copy(warmout_sb[:], wps[0:1, 0:1])

        enc4_t, encn_t = {}, {}
        probs_t, orow_t = {}, {}
        run_t = {}

        halves = [(gg * GRP + j, h) for gg in range(NGRP)
                  for h in range(NH) for j in range(GRP)]

        def issue_enc4(b, h):
            for st in (2 * h, 2 * h + 1):
                t = enc4_pool.tile([P, G, KO, ST], fp8, tag="enc4",
                                   name=f"enc4_{b}_{st}")
                # batch 0: odd block via the scalar HWDGE ring so both
                # FIFO chains deliver the first half in parallel
                eng = nc.scalar if (b == 0 and h == 0 and st == 1) else nc.sync
                eng.dma_start(t[:], enc4_d[b, st])
                enc4_t[(b, st)] = t

        def issue_encn(b, h):
            t = encn_pool.tile([P, HSC, ENC], bf16, tag="encn",
                               name=f"encn{b}_{h}")
            nc.sync.dma_start(t[:], encn_d[b, h])
            encn_t[(b, h)] = t

        def emit_energy(b, h, dc):
            eps = ps_e.tile([P, HT], fp32, tag="pse", name=f"eps{b}_{h}_{dc}")
            for st in range(HT // ST):
                blk = h * (HT // ST) + st
                for g in range(G):
                    nc.tensor.matmul(
                        eps[:, st * ST:(st + 1) * ST],
                        lhsT=we_sb[:, g, :, dc * P:(dc + 1) * P],
                        rhs=enc4_t[(b, blk)][:, g, :, :],
                        start=(g == 0), stop=(g == G - 1),
                        perf_mode=mybir.MatmulPerfMode.DoubleRow)
            t = tanh_pool.tile([P, HT], bf16, tag="tanh",
                               name=f"tanh{b}_{h}_{dc}")
            nc.scalar.activation(t[:], eps[:], AF.Tanh,
                                 bias=biasT_sb[:, dc, b:b + 1],
                                 scale=iscale_sb[:, 0:1])
            # wv multiply + running sum on DVE
            wve_t = wve_pool.tile([P, HT], bf16, tag="wve",
                                  name=f"wve{b}_{h}_{dc}")
            nc.vector.tensor_scalar_mul(wve_t[:], t[:], wv_sb[:, dc:dc + 1])
            if dc == 0:
                run_t[(b, h)] = wve_t
            else:
                nxt = wvs_pool.tile([P, HT], bf16, tag="wvs",
                                    name=f"wvs{b}_{h}_{dc}")
                nc.vector.tensor_add(nxt[:], run_t[(b, h)][:], wve_t[:])
                run_t[(b, h)] = nxt

        def emit_scores(b, h):
            asum = run_t.pop((b, h))
            scth = ps_sc.tile([P, NSC], fp32, tag="sc", name=f"scth{b}_{h}")
            for sci in range(HSC):
                nc.tensor.matmul(scth[:, sci:sci + 1],
                                 lhsT=asum[:, sci * P:(sci + 1) * P],
                                 rhs=ones_sb[:], start=True, stop=True)
            nc.scalar.activation(probs_t[b][:, h * HSC:(h + 1) * HSC],
                                 scth[:, 0:HSC], AF.Exp)
            if h == NH - 1:
                # softmax denominator at partition 32j: col-tiled ones
                # matmul into the retired scth row, reduce+recip in lane
                gg, j = b // GRP, b % GRP
                nc.tensor.matmul(scth[32 * j:32 * j + 1, 0:NSC],
                                 lhsT=ones_sb[:], rhs=probs_t[b][:, 0:NSC],
                                 tile_position=(0, 32 * j),
                                 start=True, stop=True)
                nc.vector.tensor_reduce(ssum_sb[32 * j:32 * j + 1, gg:gg + 1],
                                        scth[32 * j:32 * j + 1, 0:NSC],
                                        axis=AX.X, op=ALU.add)
                nc.vector.reciprocal(rs_sb[32 * j:32 * j + 1, gg:gg + 1],
                                     ssum_sb[32 * j:32 * j + 1, gg:gg + 1])

        def emit_span(gg, h, c):
            # 4 concurrent col-tiled M=1 matmuls: batch gg*GRP+j on PE
            # column group j, output row at psum partition 32j
            if h == 0 and c == 0:
                orow_t[gg] = ps_or.tile([P, ENC], fp32, tag="or",
                                        name=f"orow{gg}")
            orow = orow_t[gg]
            for j in range(GRP):
                b = gg * GRP + j
                cc = h * HSC + c
                nc.tensor.matmul(
                    orow[32 * j:32 * j + 1, :],
                    lhsT=probs_t[b][:, cc:cc + 1],
                    rhs=encn_t[(b, h)][:, c, :],
                    tile_position=(0, 32 * j),
                    start=(cc == 0), stop=(cc == NSC - 1))

        def emit_finalize(gg):
            orow = orow_t.pop(gg)
            ostg = stage_pool.tile([P, ENC], fp32, tag="stg",
                                   name=f"ostg{gg}")
            nc.vector.tensor_scalar_mul(ostg[:], orow[:], rs_sb[:, gg:gg + 1])
            for j in range(GRP):
                b = gg * GRP + j
                nc.gpsimd.dma_start(out_d[b:b + 1, :],
                                    ostg[32 * j:32 * j + 1, :])

        def emit_e0(i):
            b, h = halves[i]
            if h == 0:
                probs_t[b] = probs_pool.tile([P, NSC], bf16, tag="probst",
                                             name=f"probsT{b}")
            issue_encn(b, h)
            emit_energy(b, h, 0)

        # pending weighted-sum work: list of closures, 2 drained per half
        pend = []

        def drain(n):
            for _ in range(min(n, len(pend))):
                pend.pop(0)()

        issue_enc4(*halves[0])
        issue_enc4(*halves[1])
        emit_e0(0)
        for i, (b, h) in enumerate(halves):
            if i + 2 < len(halves):
                issue_enc4(*halves[i + 2])
            emit_energy(b, h, 1)
            emit_energy(b, h, 2)
            drain(1)
            emit_energy(b, h, 3)
            drain(1)
            if i + 1 < len(halves):
                emit_e0(i + 1)
            if i >= len(halves) - 2:
                # dependency-free PE filler ahead of the tail score
                # blocks: bridges the DVE-chain wait so HAM stays warm
                wfill = ps_e.tile([P, P], fp32, tag="pse", name=f"wfill{i}")
                for k in range(10):
                    nc.tensor.matmul(wfill[:], lhsT=warm_sb[:], rhs=warm_sb[:],
                                     start=(k == 0), stop=(k == 9))
                nc.vector.tensor_copy(warmout_sb[:], wfill[0:1, 0:1])
            emit_scores(b, h)
            if i % GRP == GRP - 1:
                # sub-round (gg, h) complete: queue its 8 spans (+finalize)
                gg = b // GRP
                for c in range(HSC):
                    pend.append(lambda gg=gg, h=h, c=c: emit_span(gg, h, c))
                if h == NH - 1:
                    pend.append(lambda gg=gg: emit_finalize(gg))

        # keep the PE clock-gate warm through the final exp wait, then
        # drain the last sub-round's weighted sum + finalize
        wps2 = ps_e.tile([P, P], fp32, tag="pse", name="warmps2")
        for i in range(16):
            nc.tensor.matmul(wps2[:], lhsT=warm_sb[:], rhs=warm_sb[:],
                             start=(i == 0), stop=(i == 15))
        nc.vector.tensor_copy(warmout_sb[:], wps2[0:1, 0:1])
        drain(len(pend))

    nc.compile()
    return nc


def _get_program():
    global _PROGRAM
    if _PROGRAM is None:
        _PROGRAM = _build_program()
    return _PROGRAM


def _make_in_maps(hidden, encoder_outputs, W_attn, b_attn, w_v):
    import ml_dtypes
    bf = ml_dtypes.bfloat16
    f8 = ml_dtypes.float8_e4m3fn
    W_h, W_e = W_attn[:DEC], W_attn[DEC:]
    # [G, KO, P, DEC]: contraction index e = g*256 + ko*128 + ki
    # scaled x8 to keep fp8 mantissas in the normal range
    weT = np.ascontiguousarray(
        (np.asarray(W_e) * WSCALE).reshape(G, KO, P, DEC).astype(f8))
    wv = np.ascontiguousarray(np.asarray(w_v, np.float32).reshape(DC, P).T)
    # h_proj host-side: [B, DEC]
    hproj = (np.asarray(hidden, np.float32) @ np.asarray(W_h, np.float32)
             + np.asarray(b_attn, np.float32))
    in_maps = []
    for c in range(NCORES):
        eb = np.asarray(encoder_outputs[c * BL:(c + 1) * BL])
        # [BL, NST, P, G, KO, ST]: e = g*256 + ko*128 + p, one
        # contiguous 2KB row per partition per block; scaled x4
        enc4 = np.ascontiguousarray(
            (eb * ESCALE).transpose(0, 2, 1).reshape(BL, G, KO, P, NST, ST)
            .transpose(0, 4, 3, 1, 2, 5).astype(f8))
        # [BL, NH, P, HSC, ENC]: partition p gathers s = h*HT + c*P + p
        encN = np.ascontiguousarray(
            eb.reshape(BL, NH, HSC, P, ENC).transpose(0, 1, 3, 2, 4)
            .astype(bf))
        hp = hproj[c * BL:(c + 1) * BL]          # [BL, DEC]
        biasT = np.ascontiguousarray(
            hp.T.reshape(DC, P, BL).transpose(1, 0, 2))  # [P, DC, BL]
        in_maps.append({"encT4": enc4, "encN": encN, "weT": weT,
                        "biasT": biasT, "wv": wv})
    return in_maps


def _install_trace_hooks():
    """The agent image's antenv lacks axon_hooks; recreate it from the
    ctypes NTFF profile shim in trn_agent_boot, and stub the fish-bucket
    artifact upload so the trace path stays local."""
    import sys, types
    if "antenv.axon_hooks" not in sys.modules:
        mod = types.ModuleType("antenv.axon_hooks")
        mod._hook = None
        mod.set_axon_ntff_profile_hook = lambda h: setattr(mod, "_hook", h)
        mod.get_axon_ntff_profile_hook = lambda: mod._hook
        sys.modules["antenv.axon_hooks"] = mod
        import antenv
        antenv.axon_hooks = mod
        try:
            from trn_agent_boot.trn_boot import _ntff_profile_via_ctypes
            mod._hook = _ntff_profile_via_ctypes("/opt/axon/libaxon_pjrt.so")
        except Exception as e:
            print(f"NTFF hook install failed: {e}")
    import concourse.bass_utils as bu
    bu.upload_artifacts = lambda tmpdir: f"local:{tmpdir}"


def run(hidden, encoder_outputs, W_attn, b_attn, w_v, trace=False, tmpdir=None):
    from concourse.bass_utils import run_bass_kernel_spmd
    if trace:
        _install_trace_hooks()
    nc = _get_program()
    in_maps = _make_in_maps(hidden, encoder_outputs, W_attn, b_attn, w_v)
    res = run_bass_kernel_spmd(nc, in_maps, list(range(NCORES)),
                               trace=trace, tmpdir=tmpdir)
    out = np.concatenate([np.asarray(res.results[c]["out"], np.float32)
                          for c in range(NCORES)], axis=0)
    return out, res


def kernel(hidden, encoder_outputs, W_attn, b_attn, w_v):
    out, _ = run(hidden, encoder_outputs, W_attn, b_attn, w_v)
    return out


# revision 17
# speedup vs baseline: 1.5165x; 1.0024x over previous
"""Bahdanau-attention Trainium2 kernel (data-parallel over 8 NeuronCores).

Computation (per batch row b):
    energy[s, d] = tanh(hidden[b] @ W_h + enc[b, s] @ W_e + b_attn)   [S, D]
    scores[s]    = energy[s] . w_v                                     [S]
    attn         = softmax(scores)                                     [S]
    out[b]       = sum_s attn[s] * enc[b, s]                           [E]

v11 — batch-group col-tiled weighted sum, fused DVE chain, ACT-side
softmax sums:
  - halves processed in h-major sub-rounds of GRP=4 batches; the
    weighted sum for a sub-round runs as 8 spans of 4 col-tiled
    (tile_position=(0,32j)) M=1 matmuls — 4 batches stream
    concurrently on disjoint PE column groups, each batch's output row
    accumulating at psum partition 32j of one shared orow bank.
  - energy matmuls in fp8e4m3 DoubleRow (unchanged v10 structure);
    host stages enc*4 and W_e*8 to lift fp8 denormals, compensated
    with ACT scale=1/32 on the tanh.
  - wv multiply + running sum fused into one DVE scalar_tensor_tensor
    per dc chunk (was mul+add).
  - softmax denominator: exp accum_out gives per-partition sums free
    on ACT; GPSIMD XYZWC-reduce collapses them to [1,1] at partition
    32j; DVE reciprocal in place; one tensor_scalar scale per group.
  - psum: 3x2 banks energy + 1 orow + 1 scth = 8.
  - per-half emission: E1 E2 Wspan E3 Wspan E0' S exp, bursts spread
    2 spans per half across the following sub-round.
"""

import numpy as np

B, S, ENC, DEC = 64, 2048, 512, 512
NCORES = 8
BL = B // NCORES          # batches per core
P = 128
EC = ENC // P             # 4 e-chunks
DC = DEC // P             # 4 d-chunks
ST = 512                  # matmul moving free-dim tile / DMA block
NST = S // ST             # 4 s-blocks per batch
HT = 1024                 # psum energy tile free size (one half)
NH = S // HT              # 2 halves
NSC = S // P              # 16 s-chunks for the weighted sum
HSC = HT // P             # 8 s-chunks per half
NWARM = 64                # prewarm matmuls
G = 2                     # 256-wide DoubleRow contraction chunks
KO = 2                    # k-tiles per DoubleRow matmul
GRP = 4                   # batches per weighted-sum col-tile group
NGRP = BL // GRP
WSCALE = 8.0              # host-side W_e scale (fp8 denormal lift)
ESCALE = 4.0              # host-side enc scale for the energy copy

_PROGRAM = None


def _build_program():
    import concourse.mybir as mybir
    import concourse.tile as tile
    from concourse import bacc
    from contextlib import ExitStack

    fp32 = mybir.dt.float32
    bf16 = mybir.dt.bfloat16
    fp8 = mybir.dt.float8e4
    AF = mybir.ActivationFunctionType
    ALU = mybir.AluOpType
    AX = mybir.AxisListType

    nc = bacc.Bacc("TRN2", debug=False, target_bir_lowering=False,
                   num_devices=NCORES)

    enc4_d = nc.dram_tensor("encT4", [BL, NST, P, G, KO, ST], fp8,
                            kind="ExternalInput").ap()
    we_d = nc.dram_tensor("weT", [G, KO, P, DEC], fp8,
                          kind="ExternalInput").ap()
    encn_d = nc.dram_tensor("encN", [BL, NH, P, HSC, ENC], bf16,
                            kind="ExternalInput").ap()
    biasT_d = nc.dram_tensor("biasT", [P, DC, BL], fp32,
                             kind="ExternalInput").ap()
    wv_d = nc.dram_tensor("wv", [P, DC], fp32, kind="ExternalInput").ap()
    out_d = nc.dram_tensor("out", [BL, ENC], fp32, kind="ExternalOutput").ap()

    with tile.TileContext(nc) as tc, ExitStack() as ctx:
        const = ctx.enter_context(tc.tile_pool(name="const", bufs=1))
        # 3 energy-psum bufs (6 banks) + 1 orow bank + 1 scth bank = 8
        ps_e = ctx.enter_context(tc.tile_pool(name="ps_e", bufs=3, space="PSUM"))
        ps_or = ctx.enter_context(tc.tile_pool(name="ps_or", bufs=1, space="PSUM"))
        ps_sc = ctx.enter_context(tc.tile_pool(name="ps_sc", bufs=1, space="PSUM"))
        enc4_pool = ctx.enter_context(tc.tile_pool(name="enc4p", bufs=8))
        encn_pool = ctx.enter_context(tc.tile_pool(name="encnp", bufs=8))
        tanh_pool = ctx.enter_context(tc.tile_pool(name="tanhp", bufs=8))
        wve_pool = ctx.enter_context(tc.tile_pool(name="wvep", bufs=8))
        wvs_pool = ctx.enter_context(tc.tile_pool(name="wvsp", bufs=8))
        probs_pool = ctx.enter_context(tc.tile_pool(name="probsp", bufs=8))
        stage_pool = ctx.enter_context(tc.tile_pool(name="stagep", bufs=2))

        we_sb = const.tile([P, G, KO, DEC], fp8)
        biasT_sb = const.tile([P, DC, BL], fp32)
        wv_sb = const.tile([P, DC], fp32)
        ones_sb = const.tile([P, 1], bf16)
        warm_sb = const.tile([P, P], bf16)
        warmout_sb = const.tile([1, 1], fp32)
        ssum_sb = const.tile([P, NGRP], fp32)      # denominators at 32j
        rs_sb = const.tile([P, NGRP], fp32)        # reciprocals at 32j
        iscale_sb = const.tile([P, 1], fp32)       # 1/(WSCALE*ESCALE)

        nc.vector.memset(warm_sb[:], 1.0)
        nc.vector.memset(ones_sb[:], 1.0)
        nc.vector.memset(rs_sb[:], 1.0)
        nc.vector.memset(ssum_sb[:], 1.0)
        nc.vector.memset(iscale_sb[:], 1.0 / (WSCALE * ESCALE))

        nc.sync.dma_start(we_sb[:], we_d.rearrange("g k p d -> p g k d"))
        nc.scalar.dma_start(biasT_sb[:], biasT_d)
        nc.scalar.dma_start(wv_sb[:], wv_d)

        # HAM prewarm: dummy accumulating matmuls, no data deps
        wps = ps_e.tile([P, P], fp32, tag="pse", name="warmps")
        for i in range(NWARM):
            nc.tensor.matmul(wps[:], lhsT=warm_sb[:], rhs=warm_sb[:],
                             start=(i == 0), stop=(i == NWARM - 1))
        nc.vector.tensor_copy(warmout_sb[:], wps[0:1, 0:1])

        enc4_t, encn_t = {}, {}
        probs_t, orow_t = {}, {}
        run_t = {}

        halves = [(gg * GRP + j, h) for gg in range(NGRP)
                  for h in range(NH) for j in range(GRP)]

        def issue_enc4(b, h):
            for st in (2 * h, 2 * h + 1):
                t = enc4_pool.tile([P, G, KO, ST], fp8, tag="enc4",
                                   name=f"enc4_{b}_{st}")
                if b == 0 and h == 0:
                    # pipeline-fill: stripe each tile across both HWDGE
                    # rings so the first matmul's data lands in half the
                    # time
                    hs = ST // 2
                    nc.sync.dma_start(t[:, :, :, 0:hs],
                                      enc4_d[b, st, :, :, :, 0:hs])
                    nc.scalar.dma_start(t[:, :, :, hs:ST],
                                        enc4_d[b, st, :, :, :, hs:ST])
                else:
                    nc.sync.dma_start(t[:], enc4_d[b, st])
                enc4_t[(b, st)] = t

        def issue_encn(b, h):
            t = encn_pool.tile([P, HSC, ENC], bf16, tag="encn",
                               name=f"encn{b}_{h}")
            nc.sync.dma_start(t[:], encn_d[b, h])
            encn_t[(b, h)] = t

        def emit_energy(b, h, dc):
            eps = ps_e.tile([P, HT], fp32, tag="pse", name=f"eps{b}_{h}_{dc}")
            for st in range(HT // ST):
                blk = h * (HT // ST) + st
                for g in range(G):
                    nc.tensor.matmul(
                        eps[:, st * ST:(st + 1) * ST],
                        lhsT=we_sb[:, g, :, dc * P:(dc + 1) * P],
                        rhs=enc4_t[(b, blk)][:, g, :, :],
                        start=(g == 0), stop=(g == G - 1),
                        perf_mode=mybir.MatmulPerfMode.DoubleRow)
            t = tanh_pool.tile([P, HT], bf16, tag="tanh",
                               name=f"tanh{b}_{h}_{dc}")
            nc.scalar.activation(t[:], eps[:], AF.Tanh,
                                 bias=biasT_sb[:, dc, b:b + 1],
                                 scale=iscale_sb[:, 0:1])
            # wv multiply + running sum on DVE
            wve_t = wve_pool.tile([P, HT], bf16, tag="wve",
                                  name=f"wve{b}_{h}_{dc}")
            nc.vector.tensor_scalar_mul(wve_t[:], t[:], wv_sb[:, dc:dc + 1])
            if dc == 0:
                run_t[(b, h)] = wve_t
            else:
                nxt = wvs_pool.tile([P, HT], bf16, tag="wvs",
                                    name=f"wvs{b}_{h}_{dc}")
                nc.vector.tensor_add(nxt[:], run_t[(b, h)][:], wve_t[:])
                run_t[(b, h)] = nxt

        def emit_scores(b, h):
            asum = run_t.pop((b, h))
            scth = ps_sc.tile([P, NSC], fp32, tag="sc", name=f"scth{b}_{h}")
            for sci in range(HSC):
                nc.tensor.matmul(scth[:, sci:sci + 1],
                                 lhsT=asum[:, sci * P:(sci + 1) * P],
                                 rhs=ones_sb[:], start=True, stop=True)
            nc.scalar.activation(probs_t[b][:, h * HSC:(h + 1) * HSC],
                                 scth[:, 0:HSC], AF.Exp)
            if h == NH - 1:
                # softmax denominator at partition 32j: col-tiled ones
                # matmul into the retired scth row, reduce+recip in lane
                gg, j = b // GRP, b % GRP
                nc.tensor.matmul(scth[32 * j:32 * j + 1, 0:NSC],
                                 lhsT=ones_sb[:], rhs=probs_t[b][:, 0:NSC],
                                 tile_position=(0, 32 * j),
                                 start=True, stop=True)
                nc.vector.tensor_reduce(ssum_sb[32 * j:32 * j + 1, gg:gg + 1],
                                        scth[32 * j:32 * j + 1, 0:NSC],
                                        axis=AX.X, op=ALU.add)
                nc.vector.reciprocal(rs_sb[32 * j:32 * j + 1, gg:gg + 1],
                                     ssum_sb[32 * j:32 * j + 1, gg:gg + 1])

        def emit_span(gg, h, c):
            # 4 concurrent col-tiled M=1 matmuls: batch gg*GRP+j on PE
            # column group j, output row at psum partition 32j
            if h == 0 and c == 0:
                orow_t[gg] = ps_or.tile([P, ENC], fp32, tag="or",
                                        name=f"orow{gg}")
            orow = orow_t[gg]
            for j in range(GRP):
                b = gg * GRP + j
                cc = h * HSC + c
                nc.tensor.matmul(
                    orow[32 * j:32 * j + 1, :],
                    lhsT=probs_t[b][:, cc:cc + 1],
                    rhs=encn_t[(b, h)][:, c, :],
                    tile_position=(0, 32 * j),
                    start=(cc == 0), stop=(cc == NSC - 1))

        def emit_finalize(gg):
            orow = orow_t.pop(gg)
            ostg = stage_pool.tile([P, ENC], fp32, tag="stg",
                                   name=f"ostg{gg}")
            nc.vector.tensor_scalar_mul(ostg[:], orow[:], rs_sb[:, gg:gg + 1])
            # one issue queue per row so the 4 descriptors go out in
            # parallel instead of serializing ~600ns each on one ring
            engs = [nc.gpsimd, nc.sync, nc.scalar, nc.gpsimd]
            for j in range(GRP):
                b = gg * GRP + j
                engs[j].dma_start(out_d[b:b + 1, :],
                                  ostg[32 * j:32 * j + 1, :])

        def emit_e0(i):
            b, h = halves[i]
            if h == 0:
                probs_t[b] = probs_pool.tile([P, NSC], bf16, tag="probst",
                                             name=f"probsT{b}")
            issue_encn(b, h)
            emit_energy(b, h, 0)

        # pending weighted-sum work: list of closures, 2 drained per half;
        # group finalize runs after the half's score chain so its DVE op
        # doesn't delay the asum->scores path
        pend = []
        pend_fin = []

        def drain(n):
            for _ in range(min(n, len(pend))):
                pend.pop(0)()

        def drain_fin():
            while pend_fin and not pend:
                pend_fin.pop(0)()

        issue_enc4(*halves[0])
        issue_enc4(*halves[1])
        emit_e0(0)
        for i, (b, h) in enumerate(halves):
            if i + 2 < len(halves):
                issue_enc4(*halves[i + 2])
            emit_energy(b, h, 1)
            emit_energy(b, h, 2)
            drain(1)
            emit_energy(b, h, 3)
            drain(1)
            if i + 1 < len(halves):
                emit_e0(i + 1)
            if i >= len(halves) - 2:
                # dependency-free PE filler ahead of the tail score
                # blocks: bridges the DVE-chain wait so HAM stays warm
                wfill = ps_e.tile([P, P], fp32, tag="pse", name=f"wfill{i}")
                for k in range(10):
                    nc.tensor.matmul(wfill[:], lhsT=warm_sb[:], rhs=warm_sb[:],
                                     start=(k == 0), stop=(k == 9))
                nc.vector.tensor_copy(warmout_sb[:], wfill[0:1, 0:1])
            emit_scores(b, h)
            drain_fin()
            if i % GRP == GRP - 1:
                # sub-round (gg, h) complete: queue its 8 spans (+finalize)
                gg = b // GRP
                for c in range(HSC):
                    pend.append(lambda gg=gg, h=h, c=c: emit_span(gg, h, c))
                if h == NH - 1:
                    pend_fin.append(lambda gg=gg: emit_finalize(gg))

        # keep the PE clock-gate warm through the final exp wait, then
        # drain the last sub-round's weighted sum + finalize
        wps2 = ps_e.tile([P, P], fp32, tag="pse", name="warmps2")
        for i in range(16):
            nc.tensor.matmul(wps2[:], lhsT=warm_sb[:], rhs=warm_sb[:],
                             start=(i == 0), stop=(i == 15))
        nc.vector.tensor_copy(warmout_sb[:], wps2[0:1, 0:1])
        drain(len(pend))
        drain_fin()

    nc.compile()
    return nc


def _get_program():
    global _PROGRAM
    if _PROGRAM is None:
        _PROGRAM = _build_program()
    return _PROGRAM


def _make_in_maps(hidden, encoder_outputs, W_attn, b_attn, w_v):
    import ml_dtypes
    bf = ml_dtypes.bfloat16
    f8 = ml_dtypes.float8_e4m3fn
    W_h, W_e = W_attn[:DEC], W_attn[DEC:]
    # [G, KO, P, DEC]: contraction index e = g*256 + ko*128 + ki
    # scaled x8 to keep fp8 mantissas in the normal range
    weT = np.ascontiguousarray(
        (np.asarray(W_e) * WSCALE).reshape(G, KO, P, DEC).astype(f8))
    wv = np.ascontiguousarray(np.asarray(w_v, np.float32).reshape(DC, P).T)
    # h_proj host-side: [B, DEC]
    hproj = (np.asarray(hidden, np.float32) @ np.asarray(W_h, np.float32)
             + np.asarray(b_attn, np.float32))
    in_maps = []
    for c in range(NCORES):
        eb = np.asarray(encoder_outputs[c * BL:(c + 1) * BL])
        # [BL, NST, P, G, KO, ST]: e = g*256 + ko*128 + p, one
        # contiguous 2KB row per partition per block; scaled x4
        enc4 = np.ascontiguousarray(
            (eb * ESCALE).transpose(0, 2, 1).reshape(BL, G, KO, P, NST, ST)
            .transpose(0, 4, 3, 1, 2, 5).astype(f8))
        # [BL, NH, P, HSC, ENC]: partition p gathers s = h*HT + c*P + p
        encN = np.ascontiguousarray(
            eb.reshape(BL, NH, HSC, P, ENC).transpose(0, 1, 3, 2, 4)
            .astype(bf))
        hp = hproj[c * BL:(c + 1) * BL]          # [BL, DEC]
        biasT = np.ascontiguousarray(
            hp.T.reshape(DC, P, BL).transpose(1, 0, 2))  # [P, DC, BL]
        in_maps.append({"encT4": enc4, "encN": encN, "weT": weT,
                        "biasT": biasT, "wv": wv})
    return in_maps


def _install_trace_hooks():
    """The agent image's antenv lacks axon_hooks; recreate it from the
    ctypes NTFF profile shim in trn_agent_boot, and stub the fish-bucket
    artifact upload so the trace path stays local."""
    import sys, types
    if "antenv.axon_hooks" not in sys.modules:
        mod = types.ModuleType("antenv.axon_hooks")
        mod._hook = None
        mod.set_axon_ntff_profile_hook = lambda h: setattr(mod, "_hook", h)
        mod.get_axon_ntff_profile_hook = lambda: mod._hook
        sys.modules["antenv.axon_hooks"] = mod
        import antenv
        antenv.axon_hooks = mod
        try:
            from trn_agent_boot.trn_boot import _ntff_profile_via_ctypes
            mod._hook = _ntff_profile_via_ctypes("/opt/axon/libaxon_pjrt.so")
        except Exception as e:
            print(f"NTFF hook install failed: {e}")
    import concourse.bass_utils as bu
    bu.upload_artifacts = lambda tmpdir: f"local:{tmpdir}"


def run(hidden, encoder_outputs, W_attn, b_attn, w_v, trace=False, tmpdir=None):
    from concourse.bass_utils import run_bass_kernel_spmd
    if trace:
        _install_trace_hooks()
    nc = _get_program()
    in_maps = _make_in_maps(hidden, encoder_outputs, W_attn, b_attn, w_v)
    res = run_bass_kernel_spmd(nc, in_maps, list(range(NCORES)),
                               trace=trace, tmpdir=tmpdir)
    out = np.concatenate([np.asarray(res.results[c]["out"], np.float32)
                          for c in range(NCORES)], axis=0)
    return out, res


def kernel(hidden, encoder_outputs, W_attn, b_attn, w_v):
    out, _ = run(hidden, encoder_outputs, W_attn, b_attn, w_v)
    return out
